# revision 63
# baseline (speedup 1.0000x reference)
"""Sparse (strided) attention Trainium2 Bass kernel, SPMD over 8 NeuronCores.

Problem: GPT-style attention block with a strided sparse mask
(STRIDE=128, C=8): each query sees its own 128-block (causal) plus the
last 8 columns of every preceding 128-block.

Sharding: batch (2) x head-groups (4) = 8 cores. Core c handles batch
c//4 and heads 4*(c%4) .. 4*(c%4)+3. Host transposes the input once per
batch, slices the weights per head group, and sums the 4 partial c_proj
outputs per batch (the tensor-parallel all-reduce) before adding b_proj.

Per-core device program (float32r for the projection matmuls — 1 PE
cycle/row at moving dims >= 256; float16 for the attention-path matmul
operands — 1 cycle/row at any N with a 10-bit mantissa; fp32 PSUM):
  qkT [512,2048] = Wqk.T @ XT          (q,k head-dim-major; no transposes
                                         anywhere - kT ships to DRAM d-major
                                         and the host transposes `present` k;
                                         q/k also get fp16 shadows, and the
                                         per-partition bias folds into the
                                         PSUM-evacuation copies)
  vaug [2048,260] = X @ Wv_ext          (v seq-major; a ones column is
                                         interleaved per head via the bias
                                         trick -> softmax denominators fall
                                         out of the PV matmul for free; the
                                         summary-key rows get their own
                                         17th matmul from xt's summary cols)
  attention in S^T = [keys, queries] layout, fp16 operands:
     S^T_local  = K_blk^T.T @ Q_blk     (N=128 fp16 matmuls)
     S^T_summary= Ksum^T.T  @ Q_grp     (one N=512 matmul per 4-block group)
     P^T = exp(0.125*S^T)               (ScalarE, fp16 out; no max-subtraction:
                                         scores are O(1), exp(-1e10)=0 ==
                                         exp*mask exactly)
     causal + staircase masks           (GPSIMD affine_select: both masks are
                                         affine predicates in (key, query))
     hT_aug[65,q] = Vaug.T @ P^T        (PE; row 64 = softmax denominator;
                                         a zeroing matmul first makes the
                                         accumulation scheduling-order-proof)
     hT = hT_aug[:64] * (1/denom)       (DVE reciprocal -> PE K=1 broadcast
                                         matmul -> ACT/DVE stage copy -> DVE
                                         multiply; spread so no engine
                                         saturates the attention window)
  out_partial [2048,1024] = hT.T @ Wp_slice  (f32r PE matmuls)

The input stream (xt, wqk, wv) ships in fp16 as well - halving the
HBM-bound 12MB load phase - while ktd/vaugd/hT/wp/outp stay f32r/f32.
The output projection for each 4-block group is emitted right after the
group's attention so the outp DMA streams during later groups.

Measured on the 8-core axon trn2 fixture: h rel err 3.8e-4, present rel
err 3.3e-4 vs the fp32 jax reference; TimelineSim models 102.9us/core
(354.5us for the first correct all-fp32 version; 140.6us all-f32r with
USE_FP16_ATTN=False; 127.8us with fp32 inputs).
"""

import numpy as np

import concourse.bass as bass  # noqa: F401
import concourse.mybir as mybir
import concourse.tile as tile
from concourse import bacc
from concourse.bass_utils import run_bass_kernel_spmd

F32 = mybir.dt.float32

# float32r runs the PE at 1 cycle/row (vs 4 for float32) for moving dims
# >= 256, at ~1.4e-4 relative error (HW-measured, K=1024). The BIR verifier
# requires fp32r matmul operands to be *produced* as fp32r, so every tensor
# feeding a matmul is declared with MMDT. Set False for full-precision fp32.
USE_FP32R = True
MMDT = mybir.dt.float32r if USE_FP32R else F32
# float16 for the attention-path matmul operands (q, k, P, V): the PE runs
# 2-byte dtypes at 1 cycle/row even for N=128 (vs 4 for f32r), and fp16's
# 10-bit mantissa keeps the matmul error at ~3.2e-4 (HW-measured; bf16 is
# 2.7e-3). All attention values fit fp16 range (P in [0,1], V ~ N(0,0.6),
# scores within +-25). The ktd/vaugd outputs keep separate f32r copies.
USE_FP16_ATTN = True
ATT = mybir.dt.float16 if USE_FP16_ATTN else MMDT

SEQ = 2048
EMB = 1024
NHEAD = 16
D = 64
STRIDE = 128
C = 8
BATCH = 2
NCORES = 8

NHL = 4                # heads per core
HD = NHL * D           # 256: head dims per core
NB = SEQ // STRIDE     # 16 query/key blocks
NG = 4                 # groups of 4 query blocks
VA = D + 1             # 65: v columns + ones column per head
VAW = NHL * VA         # 260: width of the augmented v tensor
SCALE = 1.0 / float(np.sqrt(D))  # 0.125

_CACHED_NC = None


def _emit(nc):
    xt_d = nc.dram_tensor("xt", [EMB, SEQ], ATT, kind="ExternalInput").ap()
    wqk_d = nc.dram_tensor("wqk", [EMB, 2 * HD], ATT, kind="ExternalInput").ap()
    wv_d = nc.dram_tensor("wv", [EMB, VAW], ATT, kind="ExternalInput").ap()
    bv_d = nc.dram_tensor("bv", [1, VAW], MMDT, kind="ExternalInput").ap()
    wp_d = nc.dram_tensor("wp", [HD, EMB], MMDT, kind="ExternalInput").ap()
    zo_d = nc.dram_tensor("zo", [65, 577], MMDT, kind="ExternalInput").ap()
    bqkt_d = nc.dram_tensor("bqkt", [2 * HD, 1], F32, kind="ExternalInput").ap()

    outp_d = nc.dram_tensor("outp", [SEQ, EMB], F32, kind="ExternalOutput").ap()
    ktd_d = nc.dram_tensor("ktd", [HD, SEQ], MMDT, kind="ExternalOutput").ap()
    vaugd_d = nc.dram_tensor("vaugd", [SEQ, VAW], MMDT, kind="ExternalOutput").ap()

    Exp = mybir.ActivationFunctionType.Exp
    Ident = mybir.ActivationFunctionType.Identity

    # DRAM views that fold the 128-row k/seq tiles into the free dimension,
    # so one big DMA fills one wide SBUF tile: sbuf[p, t, c] = dram[t*128+p, c]
    xt_v = xt_d.rearrange("(t p) s -> p t s", p=128)        # [128, 8, 2048]
    wqk_v = wqk_d.rearrange("(t p) s -> p t s", p=128)      # [128, 8, 512]
    wv_v = wv_d.rearrange("(t p) s -> p t s", p=128)        # [128, 8, 260]
    wp_v = wp_d.rearrange("(t p) s -> p t s", p=128)        # [128, 2, 1024]
    vaugd_v = vaugd_d.rearrange("(t p) s -> p t s", p=128)  # [128, 16, 260]

    with tile.TileContext(nc) as tc:
        with tc.tile_pool(name="consts", bufs=1) as consts, \
             tc.tile_pool(name="persist", bufs=1) as persist:
            # DVE memset cannot produce float32r, so the ones/zeros rows the
            # bias and zeroing matmuls need come in as a small constant.
            # Matmul operands need matching base partitions in {0, 32, 64}:
            # row 0 supplies base-0 ones, row 64 base-64 zeros|ones.
            zo = consts.tile([65, 577], MMDT, name="zo", tag="zo")
            ones_row = zo[0:1, 65:577]
            ones64 = zo[64:65, 65:577]
            zrow = zo[64:65, 0:65]
            bqkt = consts.tile([128, 4], F32, name="bqkt", tag="bqkt")
            bv = consts.tile([1, VAW], MMDT, name="bv", tag="bv")
            wp = persist.tile([128, 2 * EMB], MMDT, name="wp", tag="wp")
            qbf = [persist.tile([128, SEQ], ATT, name=f"qbf{m}", tag=f"qbf{m}")
                   for m in range(2)]
            kt = [persist.tile([128, SEQ], MMDT, name=f"kt{i}", tag=f"kt{i}")
                  for i in range(2)]
            ktbf = kt if not USE_FP16_ATTN else \
                [persist.tile([128, SEQ], ATT, name=f"ktbf{i}", tag=f"ktbf{i}")
                 for i in range(2)]
            hT = [persist.tile([128, SEQ], MMDT, name=f"ht{t}", tag=f"ht{t}")
                  for t in range(2)]
            vaug = persist.tile([128, NB * VAW], MMDT, name="vaug", tag="vaug")
            vaugbf = vaug if not USE_FP16_ATTN else \
                persist.tile([128, NB * VAW], ATT, name="vaugbf", tag="vaugbf")
            vaugsum = persist.tile([128, VAW], ATT, name="vaugsum", tag="vaugsum")
            ktsum = [persist.tile([128, 128], ATT, name=f"ktsum{i}", tag=f"ktsum{i}")
                     for i in range(2)]

            # ============ phase 1: projections ============
            with tc.tile_pool(name="inp", bufs=1) as inp:
                xt = inp.tile([128, 8 * SEQ], ATT, name="xtt", tag="xtt")
                wqk = inp.tile([128, 8 * 2 * HD], ATT, name="wqkt", tag="wqkt")
                wv = inp.tile([128, 8 * VAW], ATT, name="wvt", tag="wvt")
                # interleaved k-pair loads: after ~2.5MB the first two k-tiles
                # of both operands are resident and qkT accumulation can start
                # k-tile 0 lands in fine-grained chunks so the very first
                # matmul (which reads xt[:, 0:512] and wqk[:, 0:128]) can
                # issue after ~0.75MB instead of ~2.5MB
                nc.sync.dma_start(out=wqk[:, 0:512], in_=wqk_v[:, 0, :])
                nc.sync.dma_start(out=xt[:, 0:1024], in_=xt_v[:, 0, 0:1024])
                nc.sync.dma_start(out=xt[:, 1024:2048], in_=xt_v[:, 0, 1024:2048])
                nc.sync.dma_start(out=wqk[:, 512:1024], in_=wqk_v[:, 1, :])
                nc.sync.dma_start(out=xt[:, SEQ:2 * SEQ], in_=xt_v[:, 1, :])
                for t2 in range(1, 4):
                    nc.sync.dma_start(
                        out=xt[:, t2 * 2 * SEQ:(t2 + 1) * 2 * SEQ].rearrange(
                            "p (t s) -> p t s", s=SEQ),
                        in_=xt_v[:, t2 * 2:(t2 + 1) * 2, :])
                    nc.sync.dma_start(
                        out=wqk[:, t2 * 2 * 512:(t2 + 1) * 2 * 512].rearrange(
                            "p (t s) -> p t s", s=512),
                        in_=wqk_v[:, t2 * 2:(t2 + 1) * 2, :])
                nc.sync.dma_start(
                    out=wv.rearrange("p (t s) -> p t s", s=VAW), in_=wv_v)
                nc.sync.dma_start(out=zo, in_=zo_d)
                nc.sync.dma_start(
                    out=bqkt.unsqueeze(2),
                    in_=bqkt_d.rearrange("(m p) o -> p m o", p=128))
                nc.sync.dma_start(out=bv, in_=bv_d)
                nc.sync.dma_start(
                    out=wp.rearrange("p (t s) -> p t s", s=EMB), in_=wp_v)

                # load the ACT exp table set (~2.7us) during the input
                # stream instead of on the first attention exp
                warm = consts.tile([1, 1], F32, name="warm", tag="warm")
                nc.scalar.activation(out=warm, in_=zo[0:1, 0:1], func=Exp)

                def xts(t, lo, hi):
                    return xt[:, t * SEQ + lo:t * SEQ + hi]

                def wqks(t, lo, hi):
                    return wqk[:, t * 512 + lo:t * 512 + hi]

                with tc.tile_pool(name="ps1", bufs=2, space="PSUM") as ps1:
                    # qkT[m*128:(m+1)*128, n*512:(n+1)*512], in per-m waves
                    # with the k-loop outermost: the 4 psum banks accumulate
                    # in lockstep as the interleaved xt/wqk pairs arrive
                    for m in range(4):
                        pss_m = [ps1.tile([128, 512], F32, name=f"ps_a{n}",
                                          tag=f"ps{n}") for n in range(4)]
                        for t in range(8):
                            for n in range(4):
                                nc.tensor.matmul(
                                    pss_m[n],
                                    wqks(t, m * 128, (m + 1) * 128),
                                    xts(t, n * 512, (n + 1) * 512),
                                    start=(t == 0), stop=(t == 7))
                        # evacuate with the per-partition qk bias folded into
                        # the copy (ScalarE Identity-with-bias / DVE
                        # tensor_scalar add); q goes to bf16 only, k to f32r
                        # (for the ktd output) plus a bf16 shadow for S^T
                        bias = bqkt[:, m:m + 1]
                        for n in range(4):
                            sl = slice(n * 512, (n + 1) * 512)
                            if m < 2:
                                if n % 2 == 0:
                                    nc.scalar.activation(
                                        out=qbf[m][:, sl], in_=pss_m[n],
                                        func=Ident, bias=bias)
                                else:
                                    nc.vector.tensor_scalar_add(
                                        out=qbf[m][:, sl], in0=pss_m[n],
                                        scalar1=bias)
                            else:
                                i = m - 2
                                nc.scalar.activation(
                                    out=kt[i][:, sl], in_=pss_m[n],
                                    func=Ident, bias=bias)
                                if USE_FP16_ATTN:
                                    nc.vector.tensor_scalar_add(
                                        out=ktbf[i][:, sl], in0=pss_m[n],
                                        scalar1=bias)
                    # kT (d-major) straight out to DRAM; host transposes
                    nc.sync.dma_start(out=ktd_d[0:128, :], in_=kt[0])
                    nc.sync.dma_start(out=ktd_d[128:256, :], in_=kt[1])

                    # v (seq-major, ones-augmented)
                    for s in range(NB):
                        ps = ps1.tile([128, VAW], F32, name="ps_b",
                                      tag=f"ps{s % 4}", padded_shape=[128, 512])
                        for t in range(8):
                            nc.tensor.matmul(
                                ps, xts(t, s * 128, (s + 1) * 128),
                                wv[:, t * VAW:(t + 1) * VAW],
                                start=(t == 0), stop=False)
                        nc.tensor.matmul(
                            ps, ones_row[0:1, 0:128], bv, start=False, stop=True)
                        vsl = slice(s * VAW, (s + 1) * VAW)
                        nc.scalar.copy(out=vaug[:, vsl], in_=ps)
                        if USE_FP16_ATTN:
                            nc.vector.tensor_copy(out=vaugbf[:, vsl], in_=ps)
                    nc.sync.dma_start(
                        out=vaugd_v,
                        in_=vaug.rearrange("p (t s) -> p t s", s=VAW))

                    # vaugsum = Vaug at the summary keys (j%128 >= 120),
                    # computed directly from the summary columns of xt as a
                    # 17th v-matmul (row 8*b+c <-> key 128*b+120+c). Matmul
                    # weights need a single free dim, so gather the strided
                    # summary columns into a contiguous tile first.
                    xsum = persist.tile([128, 8 * 128], ATT, name="xsum",
                                        tag="xsum")
                    nc.vector.tensor_copy(
                        out=xsum.rearrange("p (t b c) -> p t b c", t=8, c=8),
                        in_=xt.rearrange("p (t b s) -> p t b s", t=8, s=128)
                        [:, :, :, 120:128])
                    ps = ps1.tile([128, VAW], F32, name="ps_b",
                                  tag="ps0", padded_shape=[128, 512])
                    for t in range(8):
                        nc.tensor.matmul(
                            ps, xsum[:, t * 128:(t + 1) * 128],
                            wv[:, t * VAW:(t + 1) * VAW],
                            start=(t == 0), stop=False)
                    nc.tensor.matmul(
                        ps, ones_row[0:1, 0:128], bv, start=False, stop=True)
                    nc.scalar.copy(out=vaugsum, in_=ps)

            # summary key columns of kT, gathered: column 8*b+c <-> key 128*b+120+c
            for i in range(2):
                ksrc = ktbf[i].rearrange("p (b s) -> p b s", s=128)[:, :, 120:128]
                kdst = ktsum[i].rearrange("p (b c) -> p b c", c=8)
                nc.vector.tensor_copy(out=kdst, in_=ksrc)

            # ============ phase 2: block-sparse attention (S^T layout) ============
            with tc.tile_pool(name="psl", bufs=2, space="PSUM") as psl, \
                 tc.tile_pool(name="pss", bufs=1, space="PSUM") as pss, \
                 tc.tile_pool(name="psh", bufs=2, space="PSUM") as psh, \
                 tc.tile_pool(name="psbc", bufs=1, space="PSUM") as psbc, \
                 tc.tile_pool(name="ps3", bufs=2, space="PSUM") as ps3, \
                 tc.tile_pool(name="work", bufs=4) as work, \
                 tc.tile_pool(name="small", bufs=4) as small, \
                 tc.tile_pool(name="osb", bufs=3) as osb:
                for g in range(NG):
                    for h in range(NHL):
                        ti, po = h // 2, (h % 2) * 64
                        qh = qbf[ti][po:po + 64, :]
                        kh = ktbf[ti][po:po + 64, :]
                        ksh = ktsum[ti][po:po + 64, :]
                        nmax = 8 * (4 * g + 3)
                        gl = slice(g * 512, (g + 1) * 512)
                        ps_loc = psl.tile([128, 512], F32, name="ps_loc", tag="psloc")
                        ps_sum = pss.tile([128, 512], F32, name="ps_sum", tag="pssum")
                        for j in range(4):
                            b = 4 * g + j
                            if USE_FP16_ATTN:
                                # fp16 runs 1 cyc/row at any N
                                qn = 128
                            else:
                                # f32r needs N>=256 for 1 cyc/row: score
                                # key-block b against query blocks b,b+1; the
                                # second half lands on block b+1's region and
                                # is overwritten by its real scores (WAW
                                # overlap orders the matmuls)
                                qn = 256 if j < 3 else 128
                            nc.tensor.matmul(
                                ps_loc[:, j * 128:j * 128 + qn],
                                kh[:, b * 128:(b + 1) * 128],
                                qh[:, b * 128:b * 128 + qn],
                                start=True, stop=True)
                        # summary scores for all 4 query blocks in one matmul;
                        # non-causal entries are zeroed by the staircase mask
                        nc.tensor.matmul(
                            ps_sum[0:nmax, :], ksh[:, 0:nmax], qh[:, gl],
                            start=True, stop=True)
                        pt_loc = work.tile([128, 512], ATT, name="pt_loc",
                                           tag="ptloc")
                        nc.scalar.activation(out=pt_loc, in_=ps_loc,
                                             func=Exp, scale=SCALE)
                        # causal mask: keep where qq - kk >= 0, else 0
                        ptv = pt_loc.rearrange("p (j s) -> p j s", s=128)
                        nc.gpsimd.affine_select(
                            out=ptv, in_=ptv, compare_op=mybir.AluOpType.is_ge,
                            fill=0.0, base=0, pattern=[[0, 4], [1, 128]],
                            channel_multiplier=-1)
                        pt_sum = work.tile([128, 512], ATT, name="pt_sum",
                                           tag="ptsum")
                        nc.scalar.activation(out=pt_sum[0:nmax, :],
                                             in_=ps_sum[0:nmax, :], func=Exp,
                                             scale=SCALE)
                        if True:
                            # zero the non-causal summary keys so one merged
                            # N=512 PV matmul covers all 4 blocks: key row kk
                            # is visible to block 4g+j iff kk < 32g + 8j
                            psv = pt_sum[0:nmax, :].rearrange(
                                "p (j s) -> p j s", s=128)
                            nc.gpsimd.affine_select(
                                out=psv, in_=psv,
                                compare_op=mybir.AluOpType.is_ge,
                                fill=0.0, base=32 * g - 1,
                                pattern=[[8, 4], [0, 128]],
                                channel_multiplier=-1)
                        ps_h = psh.tile([128, 512], F32, name="ps_h", tag="psh")
                        # zeroing matmul (0 x ones) covering every element the
                        # PV matmuls below touch, so their accumulation is
                        # order-independent regardless of scheduling
                        nc.tensor.matmul(
                            ps_h[0:65, 0:512], zrow, ones64,
                            start=True, stop=False, skip_group_check=True)
                        for j in range(4):
                            b = 4 * g + j
                            sl = slice(j * 128, (j + 1) * 128)
                            nc.tensor.matmul(
                                ps_h[0:65, sl],
                                vaugbf[:, b * VAW + h * VA:b * VAW + (h + 1) * VA],
                                pt_loc[:, sl],
                                start=False, stop=False, skip_group_check=True)
                        if True:
                            nc.tensor.matmul(
                                ps_h[0:65, :],
                                vaugsum[0:nmax, h * VA:(h + 1) * VA],
                                pt_sum[0:nmax, :],
                                start=False, stop=True, skip_group_check=True)
                        recip = small.tile([1, 512], MMDT, name="recip",
                                           tag="recip")
                        with nc.allow_low_precision(
                                reason="f32r reciprocal feeds an f32r matmul; "
                                       "~6e-5 rounding on 1/denom"):
                            nc.vector.reciprocal(out=recip, in_=ps_h[64:65, :])
                        # broadcast 1/denom across partitions with a K=1
                        # matmul (ones x recip) - partition-aligned with the
                        # staged hT for the DVE multiply
                        ps_bc = psbc.tile([64, 512], F32, name="ps_bc",
                                          tag="psbc")
                        nc.tensor.matmul(ps_bc, ones_row[0:1, 0:64], recip,
                                         start=True, stop=True)
                        # stage hT through SBUF so the final multiply has a
                        # single PSUM operand (DVE has one PSUM read port)
                        stage = small.tile([64, 512], F32, name="stage",
                                           tag="stage")
                        cps = nc.scalar.copy if (4 * g + h) % 2 == 0 \
                            else nc.vector.tensor_copy
                        cps(out=stage, in_=ps_h[0:64, :])
                        nc.vector.tensor_mul(
                            out=hT[ti][po:po + 64, g * 512:(g + 1) * 512],
                            in0=stage, in1=ps_bc)
                    # output projection for this group's four seq tiles -
                    # lets the outp DMA stream during later attention groups
                    for s in range(4 * g, 4 * g + 4):
                        ob = osb.tile([128, EMB], F32, name="ob", tag="osb")
                        for n in range(2):
                            ps = ps3.tile([128, 512], F32, name="ps_o",
                                          tag="ps3")
                            for t in range(2):
                                nc.tensor.matmul(
                                    ps, hT[t][:, s * 128:(s + 1) * 128],
                                    wp[:, t * EMB + n * 512:
                                       t * EMB + (n + 1) * 512],
                                    start=(t == 0), stop=(t == 1))
                            cp = nc.scalar.copy if (s + n) % 2 == 0 \
                                else nc.vector.tensor_copy
                            cp(out=ob[:, n * 512:(n + 1) * 512], in_=ps)
                        nc.sync.dma_start(
                            out=outp_d[s * 128:(s + 1) * 128, :], in_=ob)
    return nc


def get_nc():
    global _CACHED_NC
    if _CACHED_NC is None:
        nc = bacc.Bacc("TRN2", target_bir_lowering=False, debug=False,
                       num_devices=NCORES)
        _emit(nc)
        nc.compile()
        _CACHED_NC = nc
    return _CACHED_NC


def make_in_maps(inputs, w_attn, b_attn, w_proj, b_proj):
    inputs = np.asarray(inputs, np.float32)
    w_attn = np.asarray(w_attn, np.float32)
    b_attn = np.asarray(b_attn, np.float32)
    w_proj = np.asarray(w_proj, np.float32)

    # upper-triangular (key <= query) mask tile, repeated for 4 query blocks
    att_np = np.float16 if USE_FP16_ATTN else np.float32
    # ones/zeros constant rows for the bias and psum-zeroing matmuls
    zo = np.ones((65, 577), np.float32)
    zo[64, 0:65] = 0.0

    xts = [np.ascontiguousarray(inputs[b].T).astype(att_np) for b in range(BATCH)]

    in_maps = []
    for c in range(NCORES):
        b, hg = c // NHL, c % NHL
        q0 = hg * HD
        wq = w_attn[:, q0:q0 + HD]
        wk = w_attn[:, EMB + q0:EMB + q0 + HD]
        wv_raw = w_attn[:, 2 * EMB + q0:2 * EMB + q0 + HD]
        wqk = np.ascontiguousarray(
            np.concatenate([wq, wk], axis=1)).astype(att_np)
        bqkt = np.concatenate(
            [b_attn[q0:q0 + HD], b_attn[EMB + q0:EMB + q0 + HD]]
        ).reshape(2 * HD, 1).astype(np.float32)
        wv = np.zeros((EMB, VAW), att_np)
        bv = np.zeros((1, VAW), np.float32)
        for i in range(NHL):
            wv[:, i * VA:i * VA + D] = wv_raw[:, i * D:(i + 1) * D]
            bv[0, i * VA:i * VA + D] = b_attn[2 * EMB + q0 + i * D:
                                              2 * EMB + q0 + (i + 1) * D]
            bv[0, i * VA + D] = 1.0
        wp = np.ascontiguousarray(w_proj[q0:q0 + HD, :])
        in_maps.append({
            "xt": xts[b], "wqk": wqk, "wv": wv, "bqkt": bqkt, "bv": bv,
            "wp": wp, "zo": zo,
        })
    return in_maps


def assemble(results, b_proj):
    b_proj = np.asarray(b_proj, np.float32)
    h = np.zeros((BATCH, SEQ, EMB), np.float32)
    present = np.zeros((BATCH, 2, NHEAD, SEQ, D), np.float32)
    for c in range(NCORES):
        b, hg = c // NHL, c % NHL
        h[b] += results[c]["outp"]
        ktd = results[c]["ktd"]      # [256, 2048] head-dim-major
        vaugd = results[c]["vaugd"]  # [2048, 260] with ones columns
        for i in range(NHL):
            head = hg * NHL + i
            present[b, 0, head] = ktd[i * D:(i + 1) * D, :].T
            present[b, 1, head] = vaugd[:, i * VA:i * VA + D]
    h += b_proj
    return h, present


def kernel(inputs, w_attn, b_attn, w_proj, b_proj):
    nc = get_nc()
    in_maps = make_in_maps(inputs, w_attn, b_attn, w_proj, b_proj)
    res = run_bass_kernel_spmd(nc, in_maps, core_ids=list(range(NCORES)))
    return assemble(res.results, b_proj)


# revision 64
# speedup vs baseline: 1.0086x; 1.0086x over previous
"""Sparse (strided) attention Trainium2 Bass kernel, SPMD over 8 NeuronCores.

Problem: GPT-style attention block with a strided sparse mask
(STRIDE=128, C=8): each query sees its own 128-block (causal) plus the
last 8 columns of every preceding 128-block.

Sharding: batch (2) x head-groups (4) = 8 cores. Core c handles batch
c//4 and heads 4*(c%4) .. 4*(c%4)+3. Host transposes the input once per
batch, slices the weights per head group, and sums the 4 partial c_proj
outputs per batch (the tensor-parallel all-reduce) before adding b_proj.

Per-core device program (float32r for the projection matmuls — 1 PE
cycle/row at moving dims >= 256; float16 for the attention-path matmul
operands — 1 cycle/row at any N with a 10-bit mantissa; fp32 PSUM):
  qkT [512,2048] = Wqk.T @ XT          (q,k head-dim-major; no transposes
                                         anywhere - kT ships to DRAM d-major
                                         and the host transposes `present` k;
                                         q/k also get fp16 shadows, and the
                                         per-partition bias folds into the
                                         PSUM-evacuation copies)
  vaug [2048,260] = X @ Wv_ext          (v seq-major; a ones column is
                                         interleaved per head via the bias
                                         trick -> softmax denominators fall
                                         out of the PV matmul for free; the
                                         summary-key rows get their own
                                         17th matmul from xt's summary cols)
  attention in S^T = [keys, queries] layout, fp16 operands:
     S^T_local  = K_blk^T.T @ Q_blk     (N=128 fp16 matmuls)
     S^T_summary= Ksum^T.T  @ Q_grp     (one N=512 matmul per 4-block group)
     P^T = exp(0.125*S^T)               (ScalarE, fp16 out; no max-subtraction:
                                         scores are O(1), exp(-1e10)=0 ==
                                         exp*mask exactly)
     causal + staircase masks           (GPSIMD affine_select: both masks are
                                         affine predicates in (key, query))
     hT_aug[65,q] = Vaug.T @ P^T        (PE; row 64 = softmax denominator;
                                         a zeroing matmul first makes the
                                         accumulation scheduling-order-proof)
     hT = hT_aug[:64] * (1/denom)       (DVE reciprocal -> PE K=1 broadcast
                                         matmul -> ACT/DVE stage copy -> DVE
                                         multiply; spread so no engine
                                         saturates the attention window)
  out_partial [2048,1024] = hT.T @ Wp_slice  (f32r PE matmuls)

The input stream (xt, wqk, wv) ships in fp16 as well - halving the
HBM-bound 12MB load phase - while ktd/vaugd/hT/wp/outp stay f32r/f32.
The output projection for each 4-block group is emitted right after the
group's attention so the outp DMA streams during later groups.

Measured on the 8-core axon trn2 fixture: h rel err 3.8e-4, present rel
err 3.3e-4 vs the fp32 jax reference; TimelineSim models 102.9us/core
(354.5us for the first correct all-fp32 version; 140.6us all-f32r with
USE_FP16_ATTN=False; 127.8us with fp32 inputs).
"""

import numpy as np

import concourse.bass as bass  # noqa: F401
import concourse.mybir as mybir
import concourse.tile as tile
from concourse import bacc
from concourse.bass_utils import run_bass_kernel_spmd

F32 = mybir.dt.float32

# float32r runs the PE at 1 cycle/row (vs 4 for float32) for moving dims
# >= 256, at ~1.4e-4 relative error (HW-measured, K=1024). The BIR verifier
# requires fp32r matmul operands to be *produced* as fp32r, so every tensor
# feeding a matmul is declared with MMDT. Set False for full-precision fp32.
USE_FP32R = True
MMDT = mybir.dt.float32r if USE_FP32R else F32
# float16 for the attention-path matmul operands (q, k, P, V): the PE runs
# 2-byte dtypes at 1 cycle/row even for N=128 (vs 4 for f32r), and fp16's
# 10-bit mantissa keeps the matmul error at ~3.2e-4 (HW-measured; bf16 is
# 2.7e-3). All attention values fit fp16 range (P in [0,1], V ~ N(0,0.6),
# scores within +-25). The ktd/vaugd outputs keep separate f32r copies.
USE_FP16_ATTN = True
ATT = mybir.dt.float16 if USE_FP16_ATTN else MMDT

SEQ = 2048
EMB = 1024
NHEAD = 16
D = 64
STRIDE = 128
C = 8
BATCH = 2
NCORES = 8

NHL = 4                # heads per core
HD = NHL * D           # 256: head dims per core
NB = SEQ // STRIDE     # 16 query/key blocks
NG = 4                 # groups of 4 query blocks
VA = D + 1             # 65: v columns + ones column per head
VAW = NHL * VA         # 260: width of the augmented v tensor
SCALE = 1.0 / float(np.sqrt(D))  # 0.125

_CACHED_NC = None


def _emit(nc):
    xt_d = nc.dram_tensor("xt", [EMB, SEQ], ATT, kind="ExternalInput").ap()
    wqk_d = nc.dram_tensor("wqk", [EMB, 2 * HD], ATT, kind="ExternalInput").ap()
    wv_d = nc.dram_tensor("wv", [EMB, VAW], ATT, kind="ExternalInput").ap()
    bv_d = nc.dram_tensor("bv", [1, VAW], MMDT, kind="ExternalInput").ap()
    wp_d = nc.dram_tensor("wp", [HD, EMB], MMDT, kind="ExternalInput").ap()
    zo_d = nc.dram_tensor("zo", [65, 577], MMDT, kind="ExternalInput").ap()
    bqkt_d = nc.dram_tensor("bqkt", [2 * HD, 1], F32, kind="ExternalInput").ap()

    outp_d = nc.dram_tensor("outp", [SEQ, EMB], ATT, kind="ExternalOutput").ap()
    ktd_d = nc.dram_tensor("ktd", [HD, SEQ], MMDT, kind="ExternalOutput").ap()
    vaugd_d = nc.dram_tensor("vaugd", [SEQ, VAW], MMDT, kind="ExternalOutput").ap()

    Exp = mybir.ActivationFunctionType.Exp
    Ident = mybir.ActivationFunctionType.Identity

    # DRAM views that fold the 128-row k/seq tiles into the free dimension,
    # so one big DMA fills one wide SBUF tile: sbuf[p, t, c] = dram[t*128+p, c]
    xt_v = xt_d.rearrange("(t p) s -> p t s", p=128)        # [128, 8, 2048]
    wqk_v = wqk_d.rearrange("(t p) s -> p t s", p=128)      # [128, 8, 512]
    wv_v = wv_d.rearrange("(t p) s -> p t s", p=128)        # [128, 8, 260]
    wp_v = wp_d.rearrange("(t p) s -> p t s", p=128)        # [128, 2, 1024]
    vaugd_v = vaugd_d.rearrange("(t p) s -> p t s", p=128)  # [128, 16, 260]

    with tile.TileContext(nc) as tc:
        with tc.tile_pool(name="consts", bufs=1) as consts, \
             tc.tile_pool(name="persist", bufs=1) as persist:
            # DVE memset cannot produce float32r, so the ones/zeros rows the
            # bias and zeroing matmuls need come in as a small constant.
            # Matmul operands need matching base partitions in {0, 32, 64}:
            # row 0 supplies base-0 ones, row 64 base-64 zeros|ones.
            zo = consts.tile([65, 577], MMDT, name="zo", tag="zo")
            ones_row = zo[0:1, 65:577]
            ones64 = zo[64:65, 65:577]
            zrow = zo[64:65, 0:65]
            bqkt = consts.tile([128, 4], F32, name="bqkt", tag="bqkt")
            bv = consts.tile([1, VAW], MMDT, name="bv", tag="bv")
            wp = persist.tile([128, 2 * EMB], MMDT, name="wp", tag="wp")
            qbf = [persist.tile([128, SEQ], ATT, name=f"qbf{m}", tag=f"qbf{m}")
                   for m in range(2)]
            kt = [persist.tile([128, SEQ], MMDT, name=f"kt{i}", tag=f"kt{i}")
                  for i in range(2)]
            ktbf = kt if not USE_FP16_ATTN else \
                [persist.tile([128, SEQ], ATT, name=f"ktbf{i}", tag=f"ktbf{i}")
                 for i in range(2)]
            hT = [persist.tile([128, SEQ], MMDT, name=f"ht{t}", tag=f"ht{t}")
                  for t in range(2)]
            vaug = persist.tile([128, NB * VAW], MMDT, name="vaug", tag="vaug")
            vaugbf = vaug if not USE_FP16_ATTN else \
                persist.tile([128, NB * VAW], ATT, name="vaugbf", tag="vaugbf")
            vaugsum = persist.tile([128, VAW], ATT, name="vaugsum", tag="vaugsum")
            ktsum = [persist.tile([128, 128], ATT, name=f"ktsum{i}", tag=f"ktsum{i}")
                     for i in range(2)]

            # ============ phase 1: projections ============
            with tc.tile_pool(name="inp", bufs=1) as inp:
                xt = inp.tile([128, 8 * SEQ], ATT, name="xtt", tag="xtt")
                wqk = inp.tile([128, 8 * 2 * HD], ATT, name="wqkt", tag="wqkt")
                wv = inp.tile([128, 8 * VAW], ATT, name="wvt", tag="wvt")
                # interleaved k-pair loads: after ~2.5MB the first two k-tiles
                # of both operands are resident and qkT accumulation can start
                # k-tile 0 lands in fine-grained chunks so the very first
                # matmul (which reads xt[:, 0:512] and wqk[:, 0:128]) can
                # issue after ~0.75MB instead of ~2.5MB
                nc.sync.dma_start(out=wqk[:, 0:512], in_=wqk_v[:, 0, :])
                nc.sync.dma_start(out=xt[:, 0:1024], in_=xt_v[:, 0, 0:1024])
                nc.sync.dma_start(out=xt[:, 1024:2048], in_=xt_v[:, 0, 1024:2048])
                nc.sync.dma_start(out=wqk[:, 512:1024], in_=wqk_v[:, 1, :])
                nc.sync.dma_start(out=xt[:, SEQ:2 * SEQ], in_=xt_v[:, 1, :])
                for t2 in range(1, 4):
                    nc.sync.dma_start(
                        out=xt[:, t2 * 2 * SEQ:(t2 + 1) * 2 * SEQ].rearrange(
                            "p (t s) -> p t s", s=SEQ),
                        in_=xt_v[:, t2 * 2:(t2 + 1) * 2, :])
                    nc.sync.dma_start(
                        out=wqk[:, t2 * 2 * 512:(t2 + 1) * 2 * 512].rearrange(
                            "p (t s) -> p t s", s=512),
                        in_=wqk_v[:, t2 * 2:(t2 + 1) * 2, :])
                nc.sync.dma_start(
                    out=wv.rearrange("p (t s) -> p t s", s=VAW), in_=wv_v)
                nc.sync.dma_start(out=zo, in_=zo_d)
                nc.sync.dma_start(
                    out=bqkt.unsqueeze(2),
                    in_=bqkt_d.rearrange("(m p) o -> p m o", p=128))
                nc.sync.dma_start(out=bv, in_=bv_d)
                nc.sync.dma_start(
                    out=wp.rearrange("p (t s) -> p t s", s=EMB), in_=wp_v)

                # load the ACT exp table set (~2.7us) during the input
                # stream instead of on the first attention exp
                warm = consts.tile([1, 1], F32, name="warm", tag="warm")
                nc.scalar.activation(out=warm, in_=zo[0:1, 0:1], func=Exp)

                def xts(t, lo, hi):
                    return xt[:, t * SEQ + lo:t * SEQ + hi]

                def wqks(t, lo, hi):
                    return wqk[:, t * 512 + lo:t * 512 + hi]

                with tc.tile_pool(name="ps1", bufs=2, space="PSUM") as ps1:
                    # qkT[m*128:(m+1)*128, n*512:(n+1)*512], in per-m waves
                    # with the k-loop outermost: the 4 psum banks accumulate
                    # in lockstep as the interleaved xt/wqk pairs arrive
                    for m in range(4):
                        pss_m = [ps1.tile([128, 512], F32, name=f"ps_a{n}",
                                          tag=f"ps{n}") for n in range(4)]
                        for t in range(8):
                            for n in range(4):
                                nc.tensor.matmul(
                                    pss_m[n],
                                    wqks(t, m * 128, (m + 1) * 128),
                                    xts(t, n * 512, (n + 1) * 512),
                                    start=(t == 0), stop=(t == 7))
                        # evacuate with the per-partition qk bias folded into
                        # the copy (ScalarE Identity-with-bias / DVE
                        # tensor_scalar add); q goes to bf16 only, k to f32r
                        # (for the ktd output) plus a bf16 shadow for S^T
                        bias = bqkt[:, m:m + 1]
                        for n in range(4):
                            sl = slice(n * 512, (n + 1) * 512)
                            if m < 2:
                                if n % 2 == 0:
                                    nc.scalar.activation(
                                        out=qbf[m][:, sl], in_=pss_m[n],
                                        func=Ident, bias=bias)
                                else:
                                    nc.vector.tensor_scalar_add(
                                        out=qbf[m][:, sl], in0=pss_m[n],
                                        scalar1=bias)
                            else:
                                i = m - 2
                                nc.scalar.activation(
                                    out=kt[i][:, sl], in_=pss_m[n],
                                    func=Ident, bias=bias)
                                if USE_FP16_ATTN:
                                    nc.vector.tensor_scalar_add(
                                        out=ktbf[i][:, sl], in0=pss_m[n],
                                        scalar1=bias)
                    # kT (d-major) straight out to DRAM; host transposes
                    nc.sync.dma_start(out=ktd_d[0:128, :], in_=kt[0])
                    nc.sync.dma_start(out=ktd_d[128:256, :], in_=kt[1])

                    # v (seq-major, ones-augmented)
                    for s in range(NB):
                        ps = ps1.tile([128, VAW], F32, name="ps_b",
                                      tag=f"ps{s % 4}", padded_shape=[128, 512])
                        for t in range(8):
                            nc.tensor.matmul(
                                ps, xts(t, s * 128, (s + 1) * 128),
                                wv[:, t * VAW:(t + 1) * VAW],
                                start=(t == 0), stop=False)
                        nc.tensor.matmul(
                            ps, ones_row[0:1, 0:128], bv, start=False, stop=True)
                        vsl = slice(s * VAW, (s + 1) * VAW)
                        nc.scalar.copy(out=vaug[:, vsl], in_=ps)
                        if USE_FP16_ATTN:
                            nc.vector.tensor_copy(out=vaugbf[:, vsl], in_=ps)
                    nc.sync.dma_start(
                        out=vaugd_v,
                        in_=vaug.rearrange("p (t s) -> p t s", s=VAW))

                    # vaugsum = Vaug at the summary keys (j%128 >= 120),
                    # computed directly from the summary columns of xt as a
                    # 17th v-matmul (row 8*b+c <-> key 128*b+120+c). Matmul
                    # weights need a single free dim, so gather the strided
                    # summary columns into a contiguous tile first.
                    xsum = persist.tile([128, 8 * 128], ATT, name="xsum",
                                        tag="xsum")
                    nc.vector.tensor_copy(
                        out=xsum.rearrange("p (t b c) -> p t b c", t=8, c=8),
                        in_=xt.rearrange("p (t b s) -> p t b s", t=8, s=128)
                        [:, :, :, 120:128])
                    ps = ps1.tile([128, VAW], F32, name="ps_b",
                                  tag="ps0", padded_shape=[128, 512])
                    for t in range(8):
                        nc.tensor.matmul(
                            ps, xsum[:, t * 128:(t + 1) * 128],
                            wv[:, t * VAW:(t + 1) * VAW],
                            start=(t == 0), stop=False)
                    nc.tensor.matmul(
                        ps, ones_row[0:1, 0:128], bv, start=False, stop=True)
                    nc.scalar.copy(out=vaugsum, in_=ps)

            # summary key columns of kT, gathered: column 8*b+c <-> key 128*b+120+c
            for i in range(2):
                ksrc = ktbf[i].rearrange("p (b s) -> p b s", s=128)[:, :, 120:128]
                kdst = ktsum[i].rearrange("p (b c) -> p b c", c=8)
                nc.vector.tensor_copy(out=kdst, in_=ksrc)

            # ============ phase 2: block-sparse attention (S^T layout) ============
            with tc.tile_pool(name="psl", bufs=2, space="PSUM") as psl, \
                 tc.tile_pool(name="pss", bufs=1, space="PSUM") as pss, \
                 tc.tile_pool(name="psh", bufs=2, space="PSUM") as psh, \
                 tc.tile_pool(name="psbc", bufs=1, space="PSUM") as psbc, \
                 tc.tile_pool(name="ps3", bufs=2, space="PSUM") as ps3, \
                 tc.tile_pool(name="work", bufs=4) as work, \
                 tc.tile_pool(name="small", bufs=4) as small, \
                 tc.tile_pool(name="osb", bufs=3) as osb:
                for g in range(NG):
                    for h in range(NHL):
                        ti, po = h // 2, (h % 2) * 64
                        qh = qbf[ti][po:po + 64, :]
                        kh = ktbf[ti][po:po + 64, :]
                        ksh = ktsum[ti][po:po + 64, :]
                        nmax = 8 * (4 * g + 3)
                        gl = slice(g * 512, (g + 1) * 512)
                        ps_loc = psl.tile([128, 512], F32, name="ps_loc", tag="psloc")
                        ps_sum = pss.tile([128, 512], F32, name="ps_sum", tag="pssum")
                        for j in range(4):
                            b = 4 * g + j
                            if USE_FP16_ATTN:
                                # fp16 runs 1 cyc/row at any N
                                qn = 128
                            else:
                                # f32r needs N>=256 for 1 cyc/row: score
                                # key-block b against query blocks b,b+1; the
                                # second half lands on block b+1's region and
                                # is overwritten by its real scores (WAW
                                # overlap orders the matmuls)
                                qn = 256 if j < 3 else 128
                            nc.tensor.matmul(
                                ps_loc[:, j * 128:j * 128 + qn],
                                kh[:, b * 128:(b + 1) * 128],
                                qh[:, b * 128:b * 128 + qn],
                                start=True, stop=True)
                        # summary scores for all 4 query blocks in one matmul;
                        # non-causal entries are zeroed by the staircase mask
                        nc.tensor.matmul(
                            ps_sum[0:nmax, :], ksh[:, 0:nmax], qh[:, gl],
                            start=True, stop=True)
                        pt_loc = work.tile([128, 512], ATT, name="pt_loc",
                                           tag="ptloc")
                        nc.scalar.activation(out=pt_loc, in_=ps_loc,
                                             func=Exp, scale=SCALE)
                        # causal mask: keep where qq - kk >= 0, else 0
                        ptv = pt_loc.rearrange("p (j s) -> p j s", s=128)
                        nc.gpsimd.affine_select(
                            out=ptv, in_=ptv, compare_op=mybir.AluOpType.is_ge,
                            fill=0.0, base=0, pattern=[[0, 4], [1, 128]],
                            channel_multiplier=-1)
                        pt_sum = work.tile([128, 512], ATT, name="pt_sum",
                                           tag="ptsum")
                        nc.scalar.activation(out=pt_sum[0:nmax, :],
                                             in_=ps_sum[0:nmax, :], func=Exp,
                                             scale=SCALE)
                        if True:
                            # zero the non-causal summary keys so one merged
                            # N=512 PV matmul covers all 4 blocks: key row kk
                            # is visible to block 4g+j iff kk < 32g + 8j
                            psv = pt_sum[0:nmax, :].rearrange(
                                "p (j s) -> p j s", s=128)
                            nc.gpsimd.affine_select(
                                out=psv, in_=psv,
                                compare_op=mybir.AluOpType.is_ge,
                                fill=0.0, base=32 * g - 1,
                                pattern=[[8, 4], [0, 128]],
                                channel_multiplier=-1)
                        ps_h = psh.tile([128, 512], F32, name="ps_h", tag="psh")
                        # zeroing matmul (0 x ones) covering every element the
                        # PV matmuls below touch, so their accumulation is
                        # order-independent regardless of scheduling
                        nc.tensor.matmul(
                            ps_h[0:65, 0:512], zrow, ones64,
                            start=True, stop=False, skip_group_check=True)
                        for j in range(4):
                            b = 4 * g + j
                            sl = slice(j * 128, (j + 1) * 128)
                            nc.tensor.matmul(
                                ps_h[0:65, sl],
                                vaugbf[:, b * VAW + h * VA:b * VAW + (h + 1) * VA],
                                pt_loc[:, sl],
                                start=False, stop=False, skip_group_check=True)
                        if True:
                            nc.tensor.matmul(
                                ps_h[0:65, :],
                                vaugsum[0:nmax, h * VA:(h + 1) * VA],
                                pt_sum[0:nmax, :],
                                start=False, stop=True, skip_group_check=True)
                        recip = small.tile([1, 512], MMDT, name="recip",
                                           tag="recip")
                        with nc.allow_low_precision(
                                reason="f32r reciprocal feeds an f32r matmul; "
                                       "~6e-5 rounding on 1/denom"):
                            nc.vector.reciprocal(out=recip, in_=ps_h[64:65, :])
                        # broadcast 1/denom across partitions with a K=1
                        # matmul (ones x recip) - partition-aligned with the
                        # staged hT for the DVE multiply
                        ps_bc = psbc.tile([64, 512], F32, name="ps_bc",
                                          tag="psbc")
                        nc.tensor.matmul(ps_bc, ones_row[0:1, 0:64], recip,
                                         start=True, stop=True)
                        # stage hT through SBUF so the final multiply has a
                        # single PSUM operand (DVE has one PSUM read port)
                        stage = small.tile([64, 512], F32, name="stage",
                                           tag="stage")
                        cps = nc.scalar.copy if (4 * g + h) % 2 == 0 \
                            else nc.vector.tensor_copy
                        cps(out=stage, in_=ps_h[0:64, :])
                        nc.vector.tensor_mul(
                            out=hT[ti][po:po + 64, g * 512:(g + 1) * 512],
                            in0=stage, in1=ps_bc)
                    # output projection for this group's four seq tiles -
                    # lets the outp DMA stream during later attention groups
                    for s in range(4 * g, 4 * g + 4):
                        ob = osb.tile([128, EMB], ATT, name="ob", tag="osb")
                        for n in range(2):
                            ps = ps3.tile([128, 512], F32, name="ps_o",
                                          tag="ps3")
                            for t in range(2):
                                nc.tensor.matmul(
                                    ps, hT[t][:, s * 128:(s + 1) * 128],
                                    wp[:, t * EMB + n * 512:
                                       t * EMB + (n + 1) * 512],
                                    start=(t == 0), stop=(t == 1))
                            cp = nc.scalar.copy if (s + n) % 2 == 0 \
                                else nc.vector.tensor_copy
                            cp(out=ob[:, n * 512:(n + 1) * 512], in_=ps)
                        nc.sync.dma_start(
                            out=outp_d[s * 128:(s + 1) * 128, :], in_=ob)
    return nc


def get_nc():
    global _CACHED_NC
    if _CACHED_NC is None:
        nc = bacc.Bacc("TRN2", target_bir_lowering=False, debug=False,
                       num_devices=NCORES)
        _emit(nc)
        nc.compile()
        _CACHED_NC = nc
    return _CACHED_NC


def make_in_maps(inputs, w_attn, b_attn, w_proj, b_proj):
    inputs = np.asarray(inputs, np.float32)
    w_attn = np.asarray(w_attn, np.float32)
    b_attn = np.asarray(b_attn, np.float32)
    w_proj = np.asarray(w_proj, np.float32)

    # upper-triangular (key <= query) mask tile, repeated for 4 query blocks
    att_np = np.float16 if USE_FP16_ATTN else np.float32
    # ones/zeros constant rows for the bias and psum-zeroing matmuls
    zo = np.ones((65, 577), np.float32)
    zo[64, 0:65] = 0.0

    xts = [np.ascontiguousarray(inputs[b].T).astype(att_np) for b in range(BATCH)]

    in_maps = []
    for c in range(NCORES):
        b, hg = c // NHL, c % NHL
        q0 = hg * HD
        wq = w_attn[:, q0:q0 + HD]
        wk = w_attn[:, EMB + q0:EMB + q0 + HD]
        wv_raw = w_attn[:, 2 * EMB + q0:2 * EMB + q0 + HD]
        wqk = np.ascontiguousarray(
            np.concatenate([wq, wk], axis=1)).astype(att_np)
        bqkt = np.concatenate(
            [b_attn[q0:q0 + HD], b_attn[EMB + q0:EMB + q0 + HD]]
        ).reshape(2 * HD, 1).astype(np.float32)
        wv = np.zeros((EMB, VAW), att_np)
        bv = np.zeros((1, VAW), np.float32)
        for i in range(NHL):
            wv[:, i * VA:i * VA + D] = wv_raw[:, i * D:(i + 1) * D]
            bv[0, i * VA:i * VA + D] = b_attn[2 * EMB + q0 + i * D:
                                              2 * EMB + q0 + (i + 1) * D]
            bv[0, i * VA + D] = 1.0
        wp = np.ascontiguousarray(w_proj[q0:q0 + HD, :])
        in_maps.append({
            "xt": xts[b], "wqk": wqk, "wv": wv, "bqkt": bqkt, "bv": bv,
            "wp": wp, "zo": zo,
        })
    return in_maps


def assemble(results, b_proj):
    b_proj = np.asarray(b_proj, np.float32)
    h = np.zeros((BATCH, SEQ, EMB), np.float32)
    present = np.zeros((BATCH, 2, NHEAD, SEQ, D), np.float32)
    for c in range(NCORES):
        b, hg = c // NHL, c % NHL
        h[b] += results[c]["outp"]
        ktd = results[c]["ktd"]      # [256, 2048] head-dim-major
        vaugd = results[c]["vaugd"]  # [2048, 260] with ones columns
        for i in range(NHL):
            head = hg * NHL + i
            present[b, 0, head] = ktd[i * D:(i + 1) * D, :].T
            present[b, 1, head] = vaugd[:, i * VA:i * VA + D]
    h += b_proj
    return h, present


def kernel(inputs, w_attn, b_attn, w_proj, b_proj):
    nc = get_nc()
    in_maps = make_in_maps(inputs, w_attn, b_attn, w_proj, b_proj)
    res = run_bass_kernel_spmd(nc, in_maps, core_ids=list(range(NCORES)))
    return assemble(res.results, b_proj)
